# revision 2
# baseline (speedup 1.0000x reference)
"""Causal temporal attention kernel for 8 Trainium2 NeuronCores.

Reference computation (per batch b):
    qkv = x @ w_qkv + b_qkv ; split into q,k,v heads [H=16, Dh=64]
    q += pos_bias ; S = q k^T * Dh^-0.5 ; causal softmax ; out = S v
    y = concat_heads(out) @ w_out + b_out

Sharding: batch 2-way x head-group 4-way -> 8 cores. Core c = b*4 + g
computes heads 4g..4g+3 of batch b and returns the partial
y_part = concat(out_heads) @ w_out[rows of its heads]  ([T, DIM]).
Host sums the 4 partials per batch and adds b_out.

On-core layout is fully transposed so no PE transposes are needed:
    QT/KT pair tiles [128(2 heads x 64d), T], V as AV-ready lhsT chunks
    [128k, 65] (65th column = ones so the AV matmul also produces the
    softmax denominator), S^T tiles [128k, 512q] -> exp on ACT ->
    PT [128k, 512q] -> AV accumulates outT [65, 512q] in PSUM.
    Normalization: r = 1/sums broadcast across partitions with a K=1
    ones-matmul, one DVE multiply on eviction. All matmuls use float32r
    (full-rate fp32, ~1.5e-4 relative error).
"""

import sys

sys.path.insert(0, "/opt/trn_rl_repo")

from contextlib import ExitStack

import numpy as np

import concourse.bacc as bacc
import concourse.tile as tile
from concourse import mybir
from concourse.bass_utils import run_bass_kernel_spmd

F32 = mybir.dt.float32
F32R = mybir.dt.float32r
EXP = mybir.ActivationFunctionType.Exp

B, T, DIM = 2, 2048, 1024
HEADS, DH = 16, 64
HPC = 4              # heads per core
NCORES = 8
SCALE = DH ** -0.5
QT_TILES = T // 512  # 4 q-tiles of 512
KCH = T // 128       # 16 k-chunks of 128
VSTRIDE = KCH * 65   # per-head stride in v_sb


def _build_nc():
    nc = bacc.Bacc("TRN2", target_bir_lowering=False, debug=False,
                   num_devices=NCORES)
    xt_d = nc.dram_tensor("xt", [DIM, T], F32, kind="ExternalInput").ap()
    wqk_d = nc.dram_tensor("wqk", [DIM, 512], F32, kind="ExternalInput").ap()
    wv_d = nc.dram_tensor("wv", [DIM, HPC * DH], F32, kind="ExternalInput").ap()
    qb_d = nc.dram_tensor("qbias", [128, 2], F32, kind="ExternalInput").ap()
    kb_d = nc.dram_tensor("kbias", [128, 2], F32, kind="ExternalInput").ap()
    bvb_d = nc.dram_tensor("bvb", [128, HPC * DH], F32, kind="ExternalInput").ap()
    wout_d = nc.dram_tensor("wout", [2, 128, DIM], F32, kind="ExternalInput").ap()
    mask_d = nc.dram_tensor("masks", [4, 128, 512], F32, kind="ExternalInput").ap()
    y_d = nc.dram_tensor("y", [T, DIM], F32, kind="ExternalOutput").ap()

    with tile.TileContext(nc) as tc, ExitStack() as ctx:
        res = ctx.enter_context(tc.tile_pool(name="res", bufs=1))
        small = ctx.enter_context(tc.tile_pool(name="small", bufs=4))

        # ---- resident tiles ----
        wout_t = []
        for p in range(2):
            w = res.tile([128, DIM], F32R, tag=f"wout{p}", name=f"wout{p}")
            nc.sync.dma_start(w[:], wout_d[p].bitcast(F32R))
            wout_t.append(w)
        mask_t = []
        for j in range(4):
            m = res.tile([128, 512], F32R, tag=f"mask{j}", name=f"mask{j}")
            nc.sync.dma_start(m[:], mask_d[j].bitcast(F32R))
            mask_t.append(m)
        qb = res.tile([128, 2], F32, tag="qb")
        nc.sync.dma_start(qb[:], qb_d[:, :])
        kb = res.tile([128, 2], F32, tag="kb")
        nc.sync.dma_start(kb[:], kb_d[:, :])
        bvb = res.tile([128, HPC * DH], F32, tag="bvb")
        nc.sync.dma_start(bvb[:], bvb_d[:, :])

        ones_f = small.tile([128, 64], F32, tag="ones_f")
        nc.any.memset(ones_f[:], 1.0)
        ones64 = res.tile([1, 64], F32R, tag="ones64")
        nc.vector.tensor_copy(ones64[:], ones_f[0:1, :])

        qt_sb, kt_sb, outT = [], [], []
        for p in range(2):
            qt_sb.append(res.tile([128, T], F32R, tag=f"qt{p}", name=f"qt{p}"))
            kt_sb.append(res.tile([128, T], F32R, tag=f"kt{p}", name=f"kt{p}"))
            outT.append(res.tile([128, T], F32R, tag=f"outT{p}", name=f"outT{p}"))
        v_sb = res.tile([128, HPC * VSTRIDE], F32R, tag="v_sb")

        # ---- phase A: qkv projection (pools close -> SBUF/PSUM reused) ----
        with tc.tile_pool(name="phA", bufs=1) as phA, \
             tc.tile_pool(name="psA", bufs=4, space="PSUM") as psA:
            xt = []
            for c in range(8):
                t_ = phA.tile([128, T], F32R, tag=f"xt{c}", name=f"xt{c}")
                nc.sync.dma_start(t_[:], xt_d[c * 128:(c + 1) * 128, :].bitcast(F32R))
                xt.append(t_)
            wqk_t, wv_t = [], []
            for c in range(8):
                w = phA.tile([128, 512], F32R, tag=f"wqk{c}", name=f"wqk{c}")
                nc.sync.dma_start(w[:], wqk_d[c * 128:(c + 1) * 128, :].bitcast(F32R))
                wqk_t.append(w)
                w = phA.tile([128, HPC * DH], F32R, tag=f"wv{c}", name=f"wv{c}")
                nc.sync.dma_start(w[:], wv_d[c * 128:(c + 1) * 128, :].bitcast(F32R))
                wv_t.append(w)

            # QT / KT: out[128(2h x 64d), 512t] accum over 8 c-chunks
            for qk in range(2):
                for p in range(2):
                    for tt in range(QT_TILES):
                        ps = psA.tile([128, 512], F32, tag="qkps")
                        for c in range(8):
                            nc.tensor.matmul(
                                ps[:],
                                wqk_t[c][:, (qk * 2 + p) * 128:(qk * 2 + p + 1) * 128],
                                xt[c][:, tt * 512:(tt + 1) * 512],
                                start=(c == 0), stop=(c == 7))
                        dst = (qt_sb if qk == 0 else kt_sb)[p]
                        bias = (qb if qk == 0 else kb)[:, p:p + 1]
                        nc.vector.tensor_add(
                            dst[:, tt * 512:(tt + 1) * 512], ps[:],
                            bias.to_broadcast((128, 512)))

            # V: out[128t, 256d] accum over 8 c-chunks; scatter into v_sb
            for m in range(KCH):
                ps = psA.tile([128, HPC * DH], F32, tag="vps")
                for c in range(8):
                    nc.tensor.matmul(ps[:], xt[c][:, m * 128:(m + 1) * 128],
                                     wv_t[c][:], start=(c == 0), stop=(c == 7))
                for h in range(HPC):
                    off = h * VSTRIDE + m * 65
                    nc.vector.tensor_add(v_sb[:, off:off + 64],
                                         ps[:, h * DH:(h + 1) * DH],
                                         bvb[:, h * DH:(h + 1) * DH])
                    nc.vector.tensor_copy(v_sb[:, off + 64:off + 65],
                                          ones_f[:, 0:1])

        # ---- phases B/C: attention + output projection ----
        with tc.tile_pool(name="phB", bufs=4) as phB, \
             tc.tile_pool(name="psB", bufs=3, space="PSUM") as psB, \
             tc.tile_pool(name="psO", bufs=2, space="PSUM") as psO, \
             tc.tile_pool(name="psR", bufs=1, space="PSUM") as psR, \
             tc.tile_pool(name="psY", bufs=2, space="PSUM") as psY:
            for qi in range(QT_TILES):
                qs = slice(qi * 512, (qi + 1) * 512)
                nch = 4 * (qi + 1)
                for p in range(2):
                    for hl in range(2):
                        h = 2 * p + hl
                        rows = slice(hl * 64, hl * 64 + 64)
                        o_ps = psO.tile([65, 512], F32, tag="o")
                        for j in range(nch):
                            st = psB.tile([128, 512], F32, tag="st")
                            nc.tensor.matmul(
                                st[:], kt_sb[p][rows, j * 128:(j + 1) * 128],
                                qt_sb[p][rows, qs], start=True, stop=True)
                            pt = phB.tile([128, 512], F32R, tag="pt")
                            nc.scalar.activation(pt[:], st[:], EXP, scale=SCALE)
                            jp = j - 4 * qi
                            if jp >= 0:
                                nc.vector.tensor_mul(pt[:], pt[:], mask_t[jp][:])
                            off = h * VSTRIDE + j * 65
                            nc.tensor.matmul(o_ps[:], v_sb[:, off:off + 65],
                                             pt[:], start=(j == 0),
                                             stop=(j == nch - 1))
                        # normalize: outT[rows, qs] = o_ps[0:64] / o_ps[64]
                        rs_f = small.tile([1, 512], F32, tag="rs_f")
                        nc.vector.reciprocal(rs_f[:], o_ps[64:65, :])
                        rs = small.tile([1, 512], F32R, tag="rs")
                        nc.vector.tensor_copy(rs[:], rs_f[:])
                        rb_ps = psR.tile([64, 512], F32, tag="rb")
                        nc.tensor.matmul(rb_ps[:], ones64[:], rs[:],
                                         start=True, stop=True)
                        rb_sb = phB.tile([64, 512], F32, tag="rb_sb")
                        nc.vector.tensor_copy(rb_sb[:], rb_ps[:])
                        nc.vector.tensor_mul(outT[p][rows, qs], o_ps[0:64, :],
                                             rb_sb[:])
                # output projection for this q-tile
                for qc in range(4 * qi, 4 * qi + 4):
                    qcs = slice(qc * 128, (qc + 1) * 128)
                    for ct in range(2):
                        y_ps = psY.tile([128, 512], F32, tag="y")
                        for p in range(2):
                            nc.tensor.matmul(
                                y_ps[:], outT[p][:, qcs],
                                wout_t[p][:, ct * 512:(ct + 1) * 512],
                                start=(p == 0), stop=(p == 1))
                        y_sb = phB.tile([128, 512], F32, tag="y_sb")
                        nc.vector.tensor_copy(y_sb[:], y_ps[:])
                        nc.sync.dma_start(y_d[qcs, ct * 512:(ct + 1) * 512],
                                          y_sb[:])

    nc.compile()
    return nc


_NC = None


def _get_nc():
    global _NC
    if _NC is None:
        _NC = _build_nc()
    return _NC


def _host_shards(x, w_qkv, b_qkv, w_out, b_out, pos_bias):
    x = np.asarray(x, dtype=np.float32)
    w_qkv = np.asarray(w_qkv, dtype=np.float32)
    b_qkv = np.asarray(b_qkv, dtype=np.float32)
    w_out = np.asarray(w_out, dtype=np.float32)
    pos_bias = np.asarray(pos_bias, dtype=np.float32).reshape(HEADS, DH)

    wq, wk, wv = w_qkv[:, :DIM], w_qkv[:, DIM:2 * DIM], w_qkv[:, 2 * DIM:]
    bq, bk, bv = b_qkv[:DIM], b_qkv[DIM:2 * DIM], b_qkv[2 * DIM:]

    jj = np.arange(4)[:, None, None]
    dk = np.arange(128)[None, :, None]
    dq = np.arange(512)[None, None, :]
    masks = (128 * jj + dk <= dq).astype(np.float32)

    maps = []
    for core in range(NCORES):
        b, g = divmod(core, HPC)
        h0 = HPC * g
        cols = slice(h0 * DH, (h0 + HPC) * DH)          # 256 head dims
        pair_cols = [slice((h0 + 2 * p) * DH, (h0 + 2 * p + 2) * DH)
                     for p in range(2)]
        wqk_c = np.concatenate(
            [wq[:, pair_cols[0]], wq[:, pair_cols[1]],
             wk[:, pair_cols[0]], wk[:, pair_cols[1]]], axis=1)
        qbias = np.stack(
            [bq[pair_cols[p]]
             + pos_bias[h0 + 2 * p:h0 + 2 * p + 2].reshape(-1)
             for p in range(2)], axis=1)
        kbias = np.stack([bk[pair_cols[p]] for p in range(2)], axis=1)
        bvb = np.broadcast_to(bv[cols], (128, HPC * DH))
        wout_c = np.stack([w_out[pair_cols[p], :] for p in range(2)])
        maps.append({
            "xt": np.ascontiguousarray(x[b].T),
            "wqk": np.ascontiguousarray(wqk_c),
            "wv": np.ascontiguousarray(wv[:, cols]),
            "qbias": np.ascontiguousarray(qbias),
            "kbias": np.ascontiguousarray(kbias),
            "bvb": np.ascontiguousarray(bvb),
            "wout": np.ascontiguousarray(wout_c),
            "masks": masks,
        })
    return maps


def kernel(x, w_qkv, b_qkv, w_out, b_out, pos_bias, _trace=False):
    nc = _get_nc()
    in_maps = _host_shards(x, w_qkv, b_qkv, w_out, b_out, pos_bias)
    res = run_bass_kernel_spmd(nc, in_maps, list(range(NCORES)),
                               trace=_trace)
    b_out = np.asarray(b_out, dtype=np.float32)
    y = np.empty((B, T, DIM), dtype=np.float32)
    for b in range(B):
        acc = res.results[b * HPC]["y"].astype(np.float64)
        for g in range(1, HPC):
            acc = acc + res.results[b * HPC + g]["y"]
        y[b] = (acc + b_out).astype(np.float32)
    if _trace:
        kernel._last_results = res
    return y
